# revision 32
# baseline (speedup 1.0000x reference)
"""Segment-softmax GNN attention kernel for 8 Trainium2 NeuronCores.

Math (reference): latent = leaky_relu(x @ W + b, 0.2)  -> [E, 1]
                  out = scatter_softmax(latent, index) -> [E, 1]

Design (PE matvec + partition-local scans; memory-roofline bound):
  Host: stable-sort edges by destination segment; shard segment-aligned
  across 8 cores (6250 segments each => no cross-core reduction).
  Per core, segments are packed first-fit-decreasing into the 128 SBUF
  partitions (J = E_pad/128 slots each, no intra-segment padding), so
  no segment crosses a partition boundary and the softmax needs no
  cross-partition communication at all.  Edge features are stored
  TRANSPOSED and in bf16 (halves HBM traffic; rel err ~2e-3 vs the
  2e-2 budget): xsT[f, col] with col = t*EDGE_TILE + c*128 + p for
  padded position P = p*J + t*CPT + c.  Each [128,128] chunk of a tile
  is then the stationary operand of a PE matmul against W[128,1], and
  z[p, c] lands in PSUM already in partition-major padded order.
  Unused slots get a dummy column with logit -500 -> exp == 0.
  Device per core, all static APs, phases overlapped:
    A) stream xT as fused 2-tile DMAs (16KB partition lines) strictly
       alternating the two HW-DGE queues (sync/scalar saturate the
       ~330 GB/s per-core HBM port); trailing tiles go singly so both
       queues finish together; tile 0 is split into small lead minis
       so the PE starts early.  Per tile: 32 stationary-load matmuls
       (~32ns each) -> z; DVE leaky = max(z+b, 0.2z+0.2b); scalar
       engine Exp -> e kept in SBUF.
    B) per-partition segment denominators straight from SBUF: forward
       within-segment prefix scan (state = notstart*state + e) and a
       reversed max-carry scan (state = (notend*state) max fwd)
       broadcast each segment total to its slots; then
       reciprocal_approx_fast.  Split in two column windows with a
       +-HSL-slot overlap: the left ~84% runs while phase A still
       streams; only a 192-slot window remains after the last tile.
    C) out = e * recip(denom); per-half output DMAs (left overlaps A).
  Host: inverse-permute device output back to edge order.
  No max-subtraction needed: logits ~ N(0,1) so exp is safe in f32.
"""

import os
import sys

sys.path.insert(0, "/opt/trn_rl_repo")

import numpy as np
import ml_dtypes

BF16 = ml_dtypes.bfloat16

N_NODES = 50000
N_CORES = 8
SEG_PER_CORE = N_NODES // N_CORES          # 6250
D = 128
EDGE_TILE = 4096                           # edges per phase-A tile
CPT = EDGE_TILE // 128                     # 32 slots per partition per tile
XCH = 4                                    # tile-0 fast-start split
NEG_SLOPE = 0.2
HSL = 64                                   # split overlap in slots (>= max seg)
DUMMY_Z = -500.0                           # dummy-edge logit target

_compiled_cache = {}


def _build_graph(E_pad: int):
    import concourse.bacc as bacc
    import concourse.tile as tile
    from concourse import bass, mybir

    f32 = mybir.dt.float32
    bf16 = mybir.dt.bfloat16
    n_xt = E_pad // EDGE_TILE
    J = E_pad // 128                       # slots per partition

    # slot split for phase-B/C overlap: left [0, SPL), right [SPL, J).
    # The left-half DVE chain (~4.5us) must hide under the last tiles'
    # stream, so trigger ~4 tiles before the end (even: pair boundary).
    LTILE = 2 * ((n_xt - 5) // 2)          # trigger tile for the left half
    LW = LTILE * CPT                       # left scan window [0, LW)
    SPL = LW - HSL
    RW0 = SPL - HSL                        # right scan window [RW0, J)

    nc = bacc.Bacc("TRN2", target_bir_lowering=False, debug=False,
                   num_devices=N_CORES)

    xs_d = nc.dram_tensor("xst", [128, E_pad], bf16, kind="ExternalInput")
    w_d = nc.dram_tensor("wcol", [128, 1], bf16, kind="ExternalInput")
    b_d = nc.dram_tensor("bvec", [1, 1], f32, kind="ExternalInput")
    b02_d = nc.dram_tensor("b02", [1, 1], f32, kind="ExternalInput")
    nm_d = nc.dram_tensor("nmask", [128, 2, J], bf16, kind="ExternalInput")
    out_d = nc.dram_tensor("out", [E_pad, 1], f32, kind="ExternalOutput")

    AP = bass.AP
    ALU = mybir.AluOpType
    ACT = mybir.ActivationFunctionType

    def rev(ap):
        """Reversed-free-dim view of a [128, F] AP."""
        (sp, np_), (sf, nf) = ap.ap
        return AP(tensor=ap.tensor, offset=ap.offset + sf * (nf - 1),
                  ap=[[sp, np_], [-sf, nf]])

    # the HBM port saturates (~330 GB/s) with just the two HW-DGE queues
    # at 8KB partition lines; strict alternation keeps arrivals in tile
    # order.  gpsimd only carries consts/masks/outputs.
    sched = ["sync", "scalar"]

    with tile.TileContext(nc) as tc:
        with (
            tc.tile_pool(name="consts", bufs=1) as consts,
            tc.tile_pool(name="xin", bufs=6) as xin,
            tc.tile_pool(name="mini", bufs=1) as mini,
            tc.tile_pool(name="small", bufs=4) as small,
            tc.tile_pool(name="keep", bufs=1) as keep,
            tc.tile_pool(name="bwork", bufs=1) as bwork,
            tc.tile_pool(name="zp", bufs=2, space="PSUM") as zp,
        ):
            # --- constants (wb first: needed by the first matmul) ---
            wb = consts.tile([128, 1], bf16)
            nc.gpsimd.dma_start(out=wb[:], in_=w_d[:, :])
            bb = consts.tile([128, 1], f32)
            nc.gpsimd.dma_start(
                out=bb[:], in_=AP(tensor=b_d, offset=0, ap=[[0, 128], [1, 1]])
            )
            bb02 = consts.tile([128, 1], f32)
            nc.gpsimd.dma_start(
                out=bb02[:], in_=AP(tensor=b02_d, offset=0, ap=[[0, 128], [1, 1]])
            )

            nm = consts.tile([128, 2, J], bf16)
            nc.gpsimd.dma_start(out=nm[:], in_=nm_d[:, :, :])
            nsm = nm[:, 0, :]
            nem = nm[:, 1, :]

            e4_sb = keep.tile([128, J], f32)       # all exp values, SBUF-resident
            out_sb = keep.tile([128, J], f32)

            # tile 0 split into independent mini-tiles for a fast start
            # (first two extra small so the PE can begin ASAP)
            msz = [512, 512, 1024, 1024, 512, 512]
            t0q = [nc.sync, nc.scalar, nc.sync, nc.scalar, nc.scalar, nc.scalar]
            xt0 = []
            mo = 0
            for ch, csz_ in enumerate(msz):
                mt = mini.tile([128, csz_], bf16, tag=f"mini{ch}")
                t0q[ch].dma_start(
                    out=mt[:],
                    in_=AP(tensor=xs_d, offset=mo,
                           ap=[[E_pad, 128], [1, csz_]]),
                )
                xt0.append((mt, mo, csz_))
                mo += csz_
            assert mo == EDGE_TILE

            qmap = {"sync": nc.sync, "scalar": nc.scalar, "gps": nc.gpsimd}

            def process(xt, slot0, nsl):
                """matvec+leaky+exp+blocksum for nsl slots starting at slot0."""
                zt = zp.tile([128, nsl], f32, tag=f"z{nsl}")
                for c in range(nsl):
                    nc.tensor.matmul(
                        zt[:, c:c + 1],
                        xt[:, c * 128:(c + 1) * 128],
                        wb[:],
                    )
                ut = small.tile([128, nsl], f32, tag=f"u{nsl}")
                nc.vector.tensor_scalar(out=ut[:], in0=zt[:], scalar1=NEG_SLOPE,
                                        scalar2=bb02[:, 0:1], op0=ALU.mult,
                                        op1=ALU.add)
                lt = small.tile([128, nsl], f32, tag=f"l{nsl}")
                nc.vector.scalar_tensor_tensor(out=lt[:], in0=zt[:],
                                               scalar=bb[:, 0:1], in1=ut[:],
                                               op0=ALU.add, op1=ALU.max)
                et = e4_sb[:, slot0:slot0 + nsl]
                nc.scalar.activation(out=et, in_=lt[:], func=ACT.Exp)

            def seg_denom(w0, w1, d0, d1, tag):
                """Scans over slot window [w0,w1); denominators + out = e/den
                for slots [d0,d1) + output DMA."""
                wn = w1 - w0
                fwd = bwork.tile([128, wn], f32, tag=f"f{tag}")
                nc.vector.tensor_tensor_scan(
                    out=fwd[:], data0=nsm[:, w0:w1], data1=e4_sb[:, w0:w1],
                    initial=0.0, op0=ALU.mult, op1=ALU.add)
                d4 = bwork.tile([128, wn], f32, tag=f"d{tag}")
                nc.vector.tensor_tensor_scan(
                    out=rev(d4[:]), data0=rev(nem[:, w0:w1]),
                    data1=rev(fwd[:]), initial=0.0,
                    op0=ALU.mult, op1=ALU.max)
                dn = d1 - d0
                d4e = bwork.tile([128, dn], f32, tag=f"e{tag}")
                nc.vector.tensor_scalar(out=d4e[:], in0=d4[:, d0 - w0:d1 - w0],
                                        scalar1=1e-12, scalar2=None,
                                        op0=ALU.add)
                r4 = bwork.tile([128, dn], f32, tag=f"r{tag}")
                nc.vector.reciprocal_approx_fast(out=r4[:], in_=d4e[:])
                oh = out_sb[:, d0:d1]
                nc.vector.tensor_tensor(out=oh, in0=e4_sb[:, d0:d1],
                                        in1=r4[:], op=ALU.mult)
                qmap[tag].dma_start(
                    out=AP(tensor=out_d, offset=d0, ap=[[J, 128], [1, dn]]),
                    in_=oh,
                )

            # --- phase A: hoist ALL x dispatches ahead of the compute ops
            # so no engine's dispatch instruction queues behind compute
            # (in-order engines: an Exp waiting on the stream would stall
            # the next dispatch and starve that DMA queue) ---
            nsolo = 2 + (n_xt - 1) % 2             # trailing single tiles
            npair = (n_xt - 1 - nsolo) // 2
            work = [(mt[:], mo // 128, csz_ // 128) for mt, mo, csz_ in xt0]
            for k in range(npair):
                i = 1 + 2 * k
                xt = xin.tile([128, 2 * EDGE_TILE], bf16)
                qmap[sched[k % 2]].dma_start(
                    out=xt[:],
                    in_=AP(tensor=xs_d, offset=i * EDGE_TILE,
                           ap=[[E_pad, 128], [1, 2 * EDGE_TILE]]),
                )
                work.append((xt[:, 0:EDGE_TILE], i * CPT, CPT))
                work.append((xt[:, EDGE_TILE:], (i + 1) * CPT, CPT))
            for i in range(n_xt - nsolo, n_xt):
                xt = xin.tile([128, EDGE_TILE], bf16, tag="solo")
                qmap["scalar"].dma_start(
                    out=xt[:],
                    in_=AP(tensor=xs_d, offset=i * EDGE_TILE,
                           ap=[[E_pad, 128], [1, EDGE_TILE]]),
                )
                work.append((xt[:], i * CPT, CPT))

            # --- compute + overlapped left-half phase B/C ---
            for view, slot0, nsl in work:
                process(view, slot0, nsl)
                if slot0 + nsl == LTILE * CPT:
                    seg_denom(0, LW, 0, SPL, "sync")
            seg_denom(RW0, J, SPL, J, "sync")

    nc.compile()
    return nc


def _host_prep(x, W, b, index):
    """Sort/pad/bin-pack/shard on host; per-core in_maps + reassembly info."""
    x = np.ascontiguousarray(np.asarray(x, dtype=np.float32))
    W = np.asarray(W, dtype=np.float32).reshape(D)
    b = np.asarray(b, dtype=np.float32).reshape(1)
    idx = np.asarray(index).astype(np.int64).ravel()
    E = idx.shape[0]

    order = np.argsort(idx, kind="stable")
    idx_s = idx[order]
    counts = np.bincount(idx_s, minlength=N_NODES).astype(np.int64)
    seg_starts = np.zeros(N_NODES + 1, dtype=np.int64)
    np.cumsum(counts, out=seg_starts[1:])

    core_e = seg_starts[np.arange(N_CORES + 1) * SEG_PER_CORE]

    # the split-window scans assume every segment spans <= HSL slots
    assert int(counts.max()) <= HSL, f"segment length {counts.max()} > {HSL}"

    # per-core first-fit-decreasing packing of segments (no padding, G=1)
    # into 128 partitions of J slots; J grows in EDGE_TILE/128 steps
    J = 800
    packs = None
    while True:
        packs = []
        ok = True
        for k in range(N_CORES):
            s0 = k * SEG_PER_CORE
            pl = counts[s0:s0 + SEG_PER_CORE]
            sord = np.argsort(pl, kind="stable")[::-1]     # big first
            binid = np.empty(SEG_PER_CORE, dtype=np.int64)
            off = np.empty(SEG_PER_CORE, dtype=np.int64)
            rem = np.full(128, J, dtype=np.int64)
            for s in sord:
                L = int(pl[s])
                bi = int(np.argmax(rem >= L))
                if rem[bi] < L:
                    ok = False
                    break
                binid[s] = bi
                off[s] = J - rem[bi]
                rem[bi] -= L
            if not ok:
                break
            packs.append((binid, off))
        if ok:
            break
        J += EDGE_TILE // 128  # keep E_pad % EDGE_TILE == 0

    E_pad = 128 * J
    x_sorted = x[order]
    wcol = W.reshape(128, 1).astype(BF16)
    bvec = b.reshape(1, 1).astype(np.float32)
    b02 = (NEG_SLOPE * b).reshape(1, 1).astype(np.float32)
    wsq = float(W @ W)
    dummy_col = ((DUMMY_Z / max(wsq, 1e-30)) * W).astype(BF16)  # logit ~ -500

    # padded position P = p*J + t*CPT + c  ->  xsT column t*EDGE_TILE + c*128 + p
    Pv = np.arange(E_pad, dtype=np.int64)
    colmap = ((Pv % J) // CPT) * EDGE_TILE + (Pv % CPT) * 128 + (Pv // J)

    in_maps = []
    reasm = []
    for k in range(N_CORES):
        e0, e1 = int(core_e[k]), int(core_e[k + 1])
        cnt = e1 - e0
        s0 = k * SEG_PER_CORE
        binid, off = packs[k]
        sstart = seg_starts[s0:s0 + SEG_PER_CORE] - e0     # compact local starts

        seg_local = (idx_s[e0:e1] - s0).astype(np.int64)
        pos_in_seg = np.arange(cnt, dtype=np.int64) - sstart[seg_local]
        ppos = binid[seg_local] * J + off[seg_local] + pos_in_seg

        xst = np.empty((128, E_pad), dtype=BF16)
        xst[:] = dummy_col[:, None]
        xst[:, colmap[ppos]] = x_sorted[e0:e1].astype(BF16).T

        # per-slot segment id (unique ids for dummy slots)
        sseg = np.full(128 * J, -1, dtype=np.int64)
        pl = counts[s0:s0 + SEG_PER_CORE]
        slot0 = binid * J + off
        rep_seg = np.repeat(np.arange(SEG_PER_CORE), pl)
        rep_slot = np.repeat(slot0, pl) + (
            np.arange(int(pl.sum()), dtype=np.int64)
            - np.repeat(np.cumsum(pl) - pl, pl))
        sseg[rep_slot] = rep_seg
        dummy_mask = sseg < 0
        sseg[dummy_mask] = SEG_PER_CORE + np.arange(int(dummy_mask.sum()))
        sseg2 = sseg.reshape(128, J)
        notstart = np.ones((128, J), np.float32)
        notstart[:, 1:] = (sseg2[:, 1:] == sseg2[:, :-1])
        notstart[:, 0] = 0.0
        notend = np.ones((128, J), np.float32)
        notend[:, :-1] = (sseg2[:, :-1] == sseg2[:, 1:])
        notend[:, -1] = 0.0

        in_maps.append({
            "xst": xst, "wcol": wcol, "bvec": bvec, "b02": b02,
            "nmask": np.ascontiguousarray(
                np.stack([notstart, notend], axis=1)).astype(BF16),
        })
        reasm.append(ppos)

    return in_maps, reasm, order, core_e, E_pad, E


def _emulate_core(m, E_pad):
    """Numpy emulation of the device graph for one core (host-logic check)."""
    xst, wcol, bvec = m["xst"], m["wcol"], m["bvec"]
    nsm = m["nmask"][:, 0, :].astype(np.float32)
    nem = m["nmask"][:, 1, :].astype(np.float32)
    J = E_pad // 128
    z_cols = (xst.astype(np.float32).T @ wcol.astype(np.float32)).ravel()
    cv = np.arange(E_pad, dtype=np.int64)
    t, rc = cv // EDGE_TILE, cv % EDGE_TILE
    c, p = rc // 128, rc % 128
    P = p * J + t * CPT + c
    z = np.empty(E_pad, dtype=np.float32)
    z[P] = z_cols
    b0 = bvec.ravel()[0]
    zb = z + b0
    l = np.where(zb >= 0, zb, NEG_SLOPE * zb)
    e = np.exp(l).astype(np.float32).reshape(128, J)
    # emulate the split-window scans exactly as the device does
    SPL = ((J // 2) // CPT) * CPT
    LW = SPL + HSL
    RW0 = SPL - HSL
    D4 = np.zeros((128, J), np.float32)
    for (w0, w1, d0, d1) in [(0, LW, 0, SPL), (RW0, J, SPL, J)]:
        fwd = np.zeros((128, w1 - w0), np.float32)
        st = np.zeros(128, np.float32)
        for tt in range(w1 - w0):
            st = nsm[:, w0 + tt] * st + e[:, w0 + tt]
            fwd[:, tt] = st
        d4 = np.zeros((128, w1 - w0), np.float32)
        st = np.zeros(128, np.float32)
        for tt in range(w1 - w0 - 1, -1, -1):
            st = np.maximum(nem[:, w0 + tt] * st, fwd[:, tt])
            d4[:, tt] = st
        D4[:, d0:d1] = d4[:, d0 - w0:d1 - w0]
    R4 = 1.0 / (D4 + 1e-12)
    out = e * R4
    return out.reshape(-1).astype(np.float32)


LAST_RESULTS = None  # BassKernelResults from the most recent run


def kernel(x, W, b, index):
    global LAST_RESULTS
    in_maps, reasm, order, core_e, E_pad, E = _host_prep(x, W, b, index)

    if os.environ.get("KERNEL_EMULATE"):
        outs = [_emulate_core(m, E_pad) for m in in_maps]
    else:
        from concourse.bass_utils import run_bass_kernel_spmd

        if E_pad not in _compiled_cache:
            _compiled_cache[E_pad] = _build_graph(E_pad)
        nc = _compiled_cache[E_pad]
        trace = bool(os.environ.get("BASS_TRACE"))
        LAST_RESULTS = run_bass_kernel_spmd(
            nc, in_maps, list(range(N_CORES)), trace=trace,
        )
        outs = [r["out"] for r in LAST_RESULTS.results]

    out_sorted = np.empty(E, dtype=np.float32)
    for k in range(N_CORES):
        e0, e1 = int(core_e[k]), int(core_e[k + 1])
        out_sorted[e0:e1] = np.asarray(outs[k]).ravel()[reasm[k]]
    out = np.empty(E, dtype=np.float32)
    out[order] = out_sorted
    return out[:, None]


# revision 33
# speedup vs baseline: 1.0957x; 1.0957x over previous
"""Segment-softmax GNN attention kernel for 8 Trainium2 NeuronCores.

Math (reference): latent = leaky_relu(x @ W + b, 0.2)  -> [E, 1]
                  out = scatter_softmax(latent, index) -> [E, 1]

Design (PE matvec + partition-local scans; memory-roofline bound):
  Host: stable-sort edges by destination segment; shard segment-aligned
  across 8 cores (6250 segments each => no cross-core reduction).
  Per core, segments are packed first-fit-decreasing into the 128 SBUF
  partitions (J = E_pad/128 slots each, no intra-segment padding), so
  no segment crosses a partition boundary and the softmax needs no
  cross-partition communication at all.  Edge features are stored
  TRANSPOSED and in bf16 (halves HBM traffic; rel err ~2e-3 vs the
  2e-2 budget): xsT[f, col] with col = t*EDGE_TILE + c*128 + p for
  padded position P = p*J + t*CPT + c.  Each [128,128] chunk of a tile
  is then the stationary operand of a PE matmul against W[128,1], and
  z[p, c] lands in PSUM already in partition-major padded order.
  Unused slots get a dummy column with logit -500 -> exp == 0.
  Device per core, all static APs, phases overlapped:
    A) stream xT as fused 2-tile DMAs (16KB partition lines) strictly
       alternating the two HW-DGE queues (sync/scalar saturate the
       ~330 GB/s per-core HBM port); trailing tiles go singly so both
       queues finish together; tile 0 is split into small lead minis
       so the PE starts early.  Per tile: 32 stationary-load matmuls
       (~32ns each) -> z; DVE leaky = max(z+b, 0.2z+0.2b); scalar
       engine Exp -> e kept in SBUF.
    B) per-partition segment denominators straight from SBUF: forward
       within-segment prefix scan (state = notstart*state + e) and a
       reversed max-carry scan (state = (notend*state) max fwd)
       broadcast each segment total to its slots; then
       reciprocal_approx_fast.  Split in two column windows with a
       +-HSL-slot overlap: the left ~84% runs while phase A still
       streams; only a 192-slot window remains after the last tile.
    C) out = e * recip(denom); per-half output DMAs (left overlaps A).
  Host: inverse-permute device output back to edge order.
  No max-subtraction needed: logits ~ N(0,1) so exp is safe in f32.
"""

import os
import sys

sys.path.insert(0, "/opt/trn_rl_repo")

import numpy as np
import ml_dtypes

BF16 = ml_dtypes.bfloat16

N_NODES = 50000
N_CORES = 8
SEG_PER_CORE = N_NODES // N_CORES          # 6250
D = 128
EDGE_TILE = 4096                           # edges per phase-A tile
CPT = EDGE_TILE // 128                     # 32 slots per partition per tile
XCH = 4                                    # tile-0 fast-start split
NEG_SLOPE = 0.2
HSL = 64                                   # split overlap in slots (>= max seg)
DUMMY_Z = -500.0                           # dummy-edge logit target

_compiled_cache = {}


def _build_graph(E_pad: int):
    import concourse.bacc as bacc
    import concourse.tile as tile
    from concourse import bass, mybir

    f32 = mybir.dt.float32
    bf16 = mybir.dt.bfloat16
    n_xt = E_pad // EDGE_TILE
    J = E_pad // 128                       # slots per partition

    # slot split for phase-B/C overlap: left [0, SPL), right [SPL, J).
    # The left-half DVE chain (~4.5us) must hide under the last tiles'
    # stream, so trigger ~4 tiles before the end (even: pair boundary).
    LTILE = 2 * ((n_xt - 5) // 2)          # trigger tile for the left half
    LW = LTILE * CPT                       # left scan window [0, LW)
    SPL = LW - HSL
    RW0 = SPL - HSL                        # right scan window [RW0, J)

    nc = bacc.Bacc("TRN2", target_bir_lowering=False, debug=False,
                   num_devices=N_CORES)

    xs_d = nc.dram_tensor("xst", [128, E_pad], bf16, kind="ExternalInput")
    w_d = nc.dram_tensor("wcol", [128, 1], bf16, kind="ExternalInput")
    b_d = nc.dram_tensor("bvec", [1, 1], f32, kind="ExternalInput")
    b02_d = nc.dram_tensor("b02", [1, 1], f32, kind="ExternalInput")
    nm_d = nc.dram_tensor("nmask", [128, 2, J], bf16, kind="ExternalInput")
    out_d = nc.dram_tensor("out", [E_pad, 1], f32, kind="ExternalOutput")

    AP = bass.AP
    ALU = mybir.AluOpType
    ACT = mybir.ActivationFunctionType

    def rev(ap):
        """Reversed-free-dim view of a [128, F] AP."""
        (sp, np_), (sf, nf) = ap.ap
        return AP(tensor=ap.tensor, offset=ap.offset + sf * (nf - 1),
                  ap=[[sp, np_], [-sf, nf]])

    # the HBM port saturates (~330 GB/s) with just the two HW-DGE queues
    # at 8KB partition lines; strict alternation keeps arrivals in tile
    # order.  gpsimd only carries consts/masks/outputs.
    sched = ["sync", "scalar"]

    with tile.TileContext(nc) as tc:
        with (
            tc.tile_pool(name="consts", bufs=1) as consts,
            tc.tile_pool(name="xin", bufs=6) as xin,
            tc.tile_pool(name="mini", bufs=1) as mini,
            tc.tile_pool(name="small", bufs=4) as small,
            tc.tile_pool(name="keep", bufs=1) as keep,
            tc.tile_pool(name="bwork", bufs=1) as bwork,
            tc.tile_pool(name="zp", bufs=2, space="PSUM") as zp,
        ):
            # --- constants (wb first: needed by the first matmul) ---
            wb = consts.tile([128, 1], bf16)
            nc.gpsimd.dma_start(out=wb[:], in_=w_d[:, :])
            bb = consts.tile([128, 1], f32)
            nc.gpsimd.dma_start(
                out=bb[:], in_=AP(tensor=b_d, offset=0, ap=[[0, 128], [1, 1]])
            )
            bb02 = consts.tile([128, 1], f32)
            nc.gpsimd.dma_start(
                out=bb02[:], in_=AP(tensor=b02_d, offset=0, ap=[[0, 128], [1, 1]])
            )

            nm = consts.tile([128, 2, J], bf16)
            nc.gpsimd.dma_start(out=nm[:], in_=nm_d[:, :, :])
            nsm = nm[:, 0, :]
            nem = nm[:, 1, :]

            e4_sb = keep.tile([128, J], f32)       # all exp values, SBUF-resident
            out_sb = keep.tile([128, J], f32)

            # tile 0 split into independent mini-tiles for a fast start
            # (first two extra small so the PE can begin ASAP)
            msz = [512, 512, 1024, 1024, 512, 512]
            t0q = [nc.sync, nc.scalar, nc.sync, nc.scalar, nc.sync, nc.scalar]
            xt0 = []
            mo = 0
            for ch, csz_ in enumerate(msz):
                mt = mini.tile([128, csz_], bf16, tag=f"mini{ch}")
                t0q[ch].dma_start(
                    out=mt[:],
                    in_=AP(tensor=xs_d, offset=mo,
                           ap=[[E_pad, 128], [1, csz_]]),
                )
                xt0.append((mt, mo, csz_))
                mo += csz_
            assert mo == EDGE_TILE

            qmap = {"sync": nc.sync, "scalar": nc.scalar, "gps": nc.gpsimd}

            def process(xt, slot0, nsl):
                """matvec+leaky+exp+blocksum for nsl slots starting at slot0."""
                zt = zp.tile([128, nsl], f32, tag=f"z{nsl}")
                for c in range(nsl):
                    nc.tensor.matmul(
                        zt[:, c:c + 1],
                        xt[:, c * 128:(c + 1) * 128],
                        wb[:],
                    )
                ut = small.tile([128, nsl], f32, tag=f"u{nsl}")
                nc.vector.tensor_scalar(out=ut[:], in0=zt[:], scalar1=NEG_SLOPE,
                                        scalar2=bb02[:, 0:1], op0=ALU.mult,
                                        op1=ALU.add)
                lt = small.tile([128, nsl], f32, tag=f"l{nsl}")
                nc.vector.scalar_tensor_tensor(out=lt[:], in0=zt[:],
                                               scalar=bb[:, 0:1], in1=ut[:],
                                               op0=ALU.add, op1=ALU.max)
                et = e4_sb[:, slot0:slot0 + nsl]
                nc.scalar.activation(out=et, in_=lt[:], func=ACT.Exp)

            def seg_denom(w0, w1, d0, d1, tag):
                """Scans over slot window [w0,w1); denominators + out = e/den
                for slots [d0,d1) + output DMA."""
                wn = w1 - w0
                fwd = bwork.tile([128, wn], f32, tag=f"f{tag}")
                nc.vector.tensor_tensor_scan(
                    out=fwd[:], data0=nsm[:, w0:w1], data1=e4_sb[:, w0:w1],
                    initial=0.0, op0=ALU.mult, op1=ALU.add)
                d4 = bwork.tile([128, wn], f32, tag=f"d{tag}")
                nc.vector.tensor_tensor_scan(
                    out=rev(d4[:]), data0=rev(nem[:, w0:w1]),
                    data1=rev(fwd[:]), initial=0.0,
                    op0=ALU.mult, op1=ALU.max)
                dn = d1 - d0
                d4e = bwork.tile([128, dn], f32, tag=f"e{tag}")
                nc.vector.tensor_scalar(out=d4e[:], in0=d4[:, d0 - w0:d1 - w0],
                                        scalar1=1e-12, scalar2=None,
                                        op0=ALU.add)
                r4 = bwork.tile([128, dn], f32, tag=f"r{tag}")
                nc.vector.reciprocal_approx_fast(out=r4[:], in_=d4e[:])
                oh = out_sb[:, d0:d1]
                nc.vector.tensor_tensor(out=oh, in0=e4_sb[:, d0:d1],
                                        in1=r4[:], op=ALU.mult)
                qmap[tag].dma_start(
                    out=AP(tensor=out_d, offset=d0, ap=[[J, 128], [1, dn]]),
                    in_=oh,
                )

            # --- phase A: hoist ALL x dispatches ahead of the compute ops
            # so no engine's dispatch instruction queues behind compute
            # (in-order engines: an Exp waiting on the stream would stall
            # the next dispatch and starve that DMA queue) ---
            nsolo = 4 + (n_xt - 1) % 2             # trailing single tiles
            npair = (n_xt - 1 - nsolo) // 2
            work = [(mt[:], mo // 128, csz_ // 128) for mt, mo, csz_ in xt0]
            for k in range(npair):
                i = 1 + 2 * k
                xt = xin.tile([128, 2 * EDGE_TILE], bf16)
                qmap[sched[k % 2]].dma_start(
                    out=xt[:],
                    in_=AP(tensor=xs_d, offset=i * EDGE_TILE,
                           ap=[[E_pad, 128], [1, 2 * EDGE_TILE]]),
                )
                work.append((xt[:, 0:EDGE_TILE], i * CPT, CPT))
                work.append((xt[:, EDGE_TILE:], (i + 1) * CPT, CPT))
            for i in range(n_xt - nsolo, n_xt):
                xt = xin.tile([128, EDGE_TILE], bf16, tag="solo")
                qmap[sched[i % 2]].dma_start(
                    out=xt[:],
                    in_=AP(tensor=xs_d, offset=i * EDGE_TILE,
                           ap=[[E_pad, 128], [1, EDGE_TILE]]),
                )
                work.append((xt[:], i * CPT, CPT))

            # --- compute + overlapped left-half phase B/C ---
            for view, slot0, nsl in work:
                process(view, slot0, nsl)
                if slot0 + nsl == LTILE * CPT:
                    seg_denom(0, LW, 0, SPL, "sync")
            seg_denom(RW0, J, SPL, J, "sync")

    nc.compile()
    return nc


def _host_prep(x, W, b, index):
    """Sort/pad/bin-pack/shard on host; per-core in_maps + reassembly info."""
    x = np.ascontiguousarray(np.asarray(x, dtype=np.float32))
    W = np.asarray(W, dtype=np.float32).reshape(D)
    b = np.asarray(b, dtype=np.float32).reshape(1)
    idx = np.asarray(index).astype(np.int64).ravel()
    E = idx.shape[0]

    order = np.argsort(idx, kind="stable")
    idx_s = idx[order]
    counts = np.bincount(idx_s, minlength=N_NODES).astype(np.int64)
    seg_starts = np.zeros(N_NODES + 1, dtype=np.int64)
    np.cumsum(counts, out=seg_starts[1:])

    core_e = seg_starts[np.arange(N_CORES + 1) * SEG_PER_CORE]

    # the split-window scans assume every segment spans <= HSL slots
    assert int(counts.max()) <= HSL, f"segment length {counts.max()} > {HSL}"

    # per-core first-fit-decreasing packing of segments (no padding, G=1)
    # into 128 partitions of J slots; J grows in EDGE_TILE/128 steps
    J = 800
    packs = None
    while True:
        packs = []
        ok = True
        for k in range(N_CORES):
            s0 = k * SEG_PER_CORE
            pl = counts[s0:s0 + SEG_PER_CORE]
            sord = np.argsort(pl, kind="stable")[::-1]     # big first
            binid = np.empty(SEG_PER_CORE, dtype=np.int64)
            off = np.empty(SEG_PER_CORE, dtype=np.int64)
            rem = np.full(128, J, dtype=np.int64)
            for s in sord:
                L = int(pl[s])
                bi = int(np.argmax(rem >= L))
                if rem[bi] < L:
                    ok = False
                    break
                binid[s] = bi
                off[s] = J - rem[bi]
                rem[bi] -= L
            if not ok:
                break
            packs.append((binid, off))
        if ok:
            break
        J += EDGE_TILE // 128  # keep E_pad % EDGE_TILE == 0

    E_pad = 128 * J
    x_sorted = x[order]
    wcol = W.reshape(128, 1).astype(BF16)
    bvec = b.reshape(1, 1).astype(np.float32)
    b02 = (NEG_SLOPE * b).reshape(1, 1).astype(np.float32)
    wsq = float(W @ W)
    dummy_col = ((DUMMY_Z / max(wsq, 1e-30)) * W).astype(BF16)  # logit ~ -500

    # padded position P = p*J + t*CPT + c  ->  xsT column t*EDGE_TILE + c*128 + p
    Pv = np.arange(E_pad, dtype=np.int64)
    colmap = ((Pv % J) // CPT) * EDGE_TILE + (Pv % CPT) * 128 + (Pv // J)

    in_maps = []
    reasm = []
    for k in range(N_CORES):
        e0, e1 = int(core_e[k]), int(core_e[k + 1])
        cnt = e1 - e0
        s0 = k * SEG_PER_CORE
        binid, off = packs[k]
        sstart = seg_starts[s0:s0 + SEG_PER_CORE] - e0     # compact local starts

        seg_local = (idx_s[e0:e1] - s0).astype(np.int64)
        pos_in_seg = np.arange(cnt, dtype=np.int64) - sstart[seg_local]
        ppos = binid[seg_local] * J + off[seg_local] + pos_in_seg

        xst = np.empty((128, E_pad), dtype=BF16)
        xst[:] = dummy_col[:, None]
        xst[:, colmap[ppos]] = x_sorted[e0:e1].astype(BF16).T

        # per-slot segment id (unique ids for dummy slots)
        sseg = np.full(128 * J, -1, dtype=np.int64)
        pl = counts[s0:s0 + SEG_PER_CORE]
        slot0 = binid * J + off
        rep_seg = np.repeat(np.arange(SEG_PER_CORE), pl)
        rep_slot = np.repeat(slot0, pl) + (
            np.arange(int(pl.sum()), dtype=np.int64)
            - np.repeat(np.cumsum(pl) - pl, pl))
        sseg[rep_slot] = rep_seg
        dummy_mask = sseg < 0
        sseg[dummy_mask] = SEG_PER_CORE + np.arange(int(dummy_mask.sum()))
        sseg2 = sseg.reshape(128, J)
        notstart = np.ones((128, J), np.float32)
        notstart[:, 1:] = (sseg2[:, 1:] == sseg2[:, :-1])
        notstart[:, 0] = 0.0
        notend = np.ones((128, J), np.float32)
        notend[:, :-1] = (sseg2[:, :-1] == sseg2[:, 1:])
        notend[:, -1] = 0.0

        in_maps.append({
            "xst": xst, "wcol": wcol, "bvec": bvec, "b02": b02,
            "nmask": np.ascontiguousarray(
                np.stack([notstart, notend], axis=1)).astype(BF16),
        })
        reasm.append(ppos)

    return in_maps, reasm, order, core_e, E_pad, E


def _emulate_core(m, E_pad):
    """Numpy emulation of the device graph for one core (host-logic check)."""
    xst, wcol, bvec = m["xst"], m["wcol"], m["bvec"]
    nsm = m["nmask"][:, 0, :].astype(np.float32)
    nem = m["nmask"][:, 1, :].astype(np.float32)
    J = E_pad // 128
    z_cols = (xst.astype(np.float32).T @ wcol.astype(np.float32)).ravel()
    cv = np.arange(E_pad, dtype=np.int64)
    t, rc = cv // EDGE_TILE, cv % EDGE_TILE
    c, p = rc // 128, rc % 128
    P = p * J + t * CPT + c
    z = np.empty(E_pad, dtype=np.float32)
    z[P] = z_cols
    b0 = bvec.ravel()[0]
    zb = z + b0
    l = np.where(zb >= 0, zb, NEG_SLOPE * zb)
    e = np.exp(l).astype(np.float32).reshape(128, J)
    # emulate the split-window scans exactly as the device does
    SPL = ((J // 2) // CPT) * CPT
    LW = SPL + HSL
    RW0 = SPL - HSL
    D4 = np.zeros((128, J), np.float32)
    for (w0, w1, d0, d1) in [(0, LW, 0, SPL), (RW0, J, SPL, J)]:
        fwd = np.zeros((128, w1 - w0), np.float32)
        st = np.zeros(128, np.float32)
        for tt in range(w1 - w0):
            st = nsm[:, w0 + tt] * st + e[:, w0 + tt]
            fwd[:, tt] = st
        d4 = np.zeros((128, w1 - w0), np.float32)
        st = np.zeros(128, np.float32)
        for tt in range(w1 - w0 - 1, -1, -1):
            st = np.maximum(nem[:, w0 + tt] * st, fwd[:, tt])
            d4[:, tt] = st
        D4[:, d0:d1] = d4[:, d0 - w0:d1 - w0]
    R4 = 1.0 / (D4 + 1e-12)
    out = e * R4
    return out.reshape(-1).astype(np.float32)


LAST_RESULTS = None  # BassKernelResults from the most recent run


def kernel(x, W, b, index):
    global LAST_RESULTS
    in_maps, reasm, order, core_e, E_pad, E = _host_prep(x, W, b, index)

    if os.environ.get("KERNEL_EMULATE"):
        outs = [_emulate_core(m, E_pad) for m in in_maps]
    else:
        from concourse.bass_utils import run_bass_kernel_spmd

        if E_pad not in _compiled_cache:
            _compiled_cache[E_pad] = _build_graph(E_pad)
        nc = _compiled_cache[E_pad]
        trace = bool(os.environ.get("BASS_TRACE"))
        LAST_RESULTS = run_bass_kernel_spmd(
            nc, in_maps, list(range(N_CORES)), trace=trace,
        )
        outs = [r["out"] for r in LAST_RESULTS.results]

    out_sorted = np.empty(E, dtype=np.float32)
    for k in range(N_CORES):
        e0, e1 = int(core_e[k]), int(core_e[k + 1])
        out_sorted[e0:e1] = np.asarray(outs[k]).ravel()[reasm[k]]
    out = np.empty(E, dtype=np.float32)
    out[order] = out_sorted
    return out[:, None]


# revision 34
# speedup vs baseline: 1.1045x; 1.0080x over previous
"""Segment-softmax GNN attention kernel for 8 Trainium2 NeuronCores.

Math (reference): latent = leaky_relu(x @ W + b, 0.2)  -> [E, 1]
                  out = scatter_softmax(latent, index) -> [E, 1]

Design (PE matvec + partition-local scans; memory-roofline bound):
  Host: stable-sort edges by destination segment; shard segment-aligned
  across 8 cores (6250 segments each => no cross-core reduction).
  Per core, segments are packed first-fit-decreasing into the 128 SBUF
  partitions (J = E_pad/128 slots each, no intra-segment padding), so
  no segment crosses a partition boundary and the softmax needs no
  cross-partition communication at all.  Edge features are stored
  TRANSPOSED and in bf16 (halves HBM traffic; rel err ~2e-3 vs the
  2e-2 budget): xsT[f, col] with col = t*EDGE_TILE + c*128 + p for
  padded position P = p*J + t*CPT + c.  Each [128,128] chunk of a tile
  is then the stationary operand of a PE matmul against W[128,1], and
  z[p, c] lands in PSUM already in partition-major padded order.
  Unused slots get a dummy column with logit -500 -> exp == 0.
  Device per core, all static APs, phases overlapped:
    A) stream xT as fused 2-tile DMAs (16KB partition lines) strictly
       alternating the two HW-DGE queues (sync/scalar saturate the
       ~330 GB/s per-core HBM port); trailing tiles go singly so both
       queues finish together; tile 0 is split into small lead minis
       so the PE starts early.  Per tile: 32 stationary-load matmuls
       (~32ns each) -> z; DVE leaky = max(z+b, 0.2z+0.2b); scalar
       engine Exp -> e kept in SBUF.
    B) per-partition segment denominators straight from SBUF: forward
       within-segment prefix scan (state = notstart*state + e) and a
       reversed max-carry scan (state = (notend*state) max fwd)
       broadcast each segment total to its slots; then
       reciprocal_approx_fast.  Split in two column windows with a
       +-HSL-slot overlap: the left ~84% runs while phase A still
       streams; only a 192-slot window remains after the last tile.
    C) out = e * recip(denom); per-half output DMAs (left overlaps A).
  Host: inverse-permute device output back to edge order.
  No max-subtraction needed: logits ~ N(0,1) so exp is safe in f32.
"""

import os
import sys

sys.path.insert(0, "/opt/trn_rl_repo")

import numpy as np
import ml_dtypes

BF16 = ml_dtypes.bfloat16

N_NODES = 50000
N_CORES = 8
SEG_PER_CORE = N_NODES // N_CORES          # 6250
D = 128
EDGE_TILE = 4096                           # edges per phase-A tile
CPT = EDGE_TILE // 128                     # 32 slots per partition per tile
XCH = 4                                    # tile-0 fast-start split
NEG_SLOPE = 0.2
HSL = 64                                   # split overlap in slots (>= max seg)
DUMMY_Z = -500.0                           # dummy-edge logit target

_compiled_cache = {}


def _build_graph(E_pad: int):
    import concourse.bacc as bacc
    import concourse.tile as tile
    from concourse import bass, mybir

    f32 = mybir.dt.float32
    bf16 = mybir.dt.bfloat16
    n_xt = E_pad // EDGE_TILE
    J = E_pad // 128                       # slots per partition

    # slot split for phase-B/C overlap: left [0, SPL), right [SPL, J).
    # The left-half DVE chain (~4.5us) must hide under the last tiles'
    # stream, so trigger ~4 tiles before the end (even: pair boundary).
    LTILE = 2 * ((n_xt - 5) // 2)          # trigger tile for the left half
    LW = LTILE * CPT                       # left scan window [0, LW)
    SPL = LW - HSL
    RW0 = SPL - HSL                        # right scan window [RW0, J)

    nc = bacc.Bacc("TRN2", target_bir_lowering=False, debug=False,
                   num_devices=N_CORES)

    xs_d = nc.dram_tensor("xst", [128, E_pad], bf16, kind="ExternalInput")
    w_d = nc.dram_tensor("wcol", [128, 1], bf16, kind="ExternalInput")
    b_d = nc.dram_tensor("bvec", [1, 1], f32, kind="ExternalInput")
    b02_d = nc.dram_tensor("b02", [1, 1], f32, kind="ExternalInput")
    nm_d = nc.dram_tensor("nmask", [128, 2, J], bf16, kind="ExternalInput")
    out_d = nc.dram_tensor("out", [E_pad, 1], f32, kind="ExternalOutput")

    AP = bass.AP
    ALU = mybir.AluOpType
    ACT = mybir.ActivationFunctionType

    def rev(ap):
        """Reversed-free-dim view of a [128, F] AP."""
        (sp, np_), (sf, nf) = ap.ap
        return AP(tensor=ap.tensor, offset=ap.offset + sf * (nf - 1),
                  ap=[[sp, np_], [-sf, nf]])

    # the HBM port saturates (~330 GB/s) with just the two HW-DGE queues
    # at 8KB partition lines; strict alternation keeps arrivals in tile
    # order.  gpsimd only carries consts/masks/outputs.
    sched = ["sync", "scalar"]

    with tile.TileContext(nc) as tc:
        with (
            tc.tile_pool(name="consts", bufs=1) as consts,
            tc.tile_pool(name="xin", bufs=5) as xin,
            tc.tile_pool(name="small", bufs=4) as small,
            tc.tile_pool(name="keep", bufs=1) as keep,
            tc.tile_pool(name="bwork", bufs=1) as bwork,
            tc.tile_pool(name="zp", bufs=2, space="PSUM") as zp,
        ):
            # --- constants (wb first: needed by the first matmul) ---
            wb = consts.tile([128, 1], bf16)
            nc.gpsimd.dma_start(out=wb[:], in_=w_d[:, :])
            bb = consts.tile([128, 1], f32)
            nc.gpsimd.dma_start(
                out=bb[:], in_=AP(tensor=b_d, offset=0, ap=[[0, 128], [1, 1]])
            )
            bb02 = consts.tile([128, 1], f32)
            nc.gpsimd.dma_start(
                out=bb02[:], in_=AP(tensor=b02_d, offset=0, ap=[[0, 128], [1, 1]])
            )

            nm = consts.tile([128, 2, J], bf16)
            nc.gpsimd.dma_start(out=nm[:], in_=nm_d[:, :, :])
            nsm = nm[:, 0, :]
            nem = nm[:, 1, :]

            e4_sb = keep.tile([128, J], f32)       # all exp values, SBUF-resident
            out_sb = keep.tile([128, J], f32)

            qmap = {"sync": nc.sync, "scalar": nc.scalar, "gps": nc.gpsimd}

            def process(xt, slot0, nsl):
                """matvec+leaky+exp+blocksum for nsl slots starting at slot0."""
                zt = zp.tile([128, nsl], f32, tag=f"z{nsl}")
                for c in range(nsl):
                    nc.tensor.matmul(
                        zt[:, c:c + 1],
                        xt[:, c * 128:(c + 1) * 128],
                        wb[:],
                    )
                ut = small.tile([128, nsl], f32, tag=f"u{nsl}")
                nc.vector.tensor_scalar(out=ut[:], in0=zt[:], scalar1=NEG_SLOPE,
                                        scalar2=bb02[:, 0:1], op0=ALU.mult,
                                        op1=ALU.add)
                lt = small.tile([128, nsl], f32, tag=f"l{nsl}")
                nc.vector.scalar_tensor_tensor(out=lt[:], in0=zt[:],
                                               scalar=bb[:, 0:1], in1=ut[:],
                                               op0=ALU.add, op1=ALU.max)
                et = e4_sb[:, slot0:slot0 + nsl]
                nc.scalar.activation(out=et, in_=lt[:], func=ACT.Exp)

            def seg_denom(w0, w1, d0, d1, tag):
                """Scans over slot window [w0,w1); denominators + out = e/den
                for slots [d0,d1) + output DMA."""
                wn = w1 - w0
                fwd = bwork.tile([128, wn], f32, tag=f"f{tag}")
                nc.vector.tensor_tensor_scan(
                    out=fwd[:], data0=nsm[:, w0:w1], data1=e4_sb[:, w0:w1],
                    initial=0.0, op0=ALU.mult, op1=ALU.add)
                d4 = bwork.tile([128, wn], f32, tag=f"d{tag}")
                nc.vector.tensor_tensor_scan(
                    out=rev(d4[:]), data0=rev(nem[:, w0:w1]),
                    data1=rev(fwd[:]), initial=0.0,
                    op0=ALU.mult, op1=ALU.max)
                dn = d1 - d0
                d4e = bwork.tile([128, dn], f32, tag=f"e{tag}")
                nc.vector.tensor_scalar(out=d4e[:], in0=d4[:, d0 - w0:d1 - w0],
                                        scalar1=1e-12, scalar2=None,
                                        op0=ALU.add)
                r4 = bwork.tile([128, dn], f32, tag=f"r{tag}")
                nc.vector.reciprocal_approx_fast(out=r4[:], in_=d4e[:])
                oh = out_sb[:, d0:d1]
                nc.vector.tensor_tensor(out=oh, in0=e4_sb[:, d0:d1],
                                        in1=r4[:], op=ALU.mult)
                qmap[tag].dma_start(
                    out=AP(tensor=out_d, offset=d0, ap=[[J, 128], [1, dn]]),
                    in_=oh,
                )

            # --- phase A: hoist ALL x dispatches ahead of the compute ops
            # so no engine's dispatch instruction queues behind compute
            # (in-order engines: an Exp waiting on the stream would stall
            # the next dispatch and starve that DMA queue).  The PE is
            # never the bottleneck, so tiles go as fused TRIPLES (24KB
            # partition lines) for maximum DMA-queue efficiency, plus
            # trailing solos; the two HW queues alternate and end even.
            ntri = n_xt // 3
            nsolo = n_xt % 3
            qb = {"sync": 0.0, "scalar": 0.0}
            work = []
            for k in range(ntri):
                i = 3 * k
                q = sched[k % 2]
                qb[q] += 3.0
                xt = xin.tile([128, 3 * EDGE_TILE], bf16)
                qmap[q].dma_start(
                    out=xt[:],
                    in_=AP(tensor=xs_d, offset=i * EDGE_TILE,
                           ap=[[E_pad, 128], [1, 3 * EDGE_TILE]]),
                )
                for t in range(3):
                    work.append((xt[:, t * EDGE_TILE:(t + 1) * EDGE_TILE],
                                 (i + t) * CPT, CPT))
            for i in range(3 * ntri, n_xt):
                q = min(qb, key=qb.get)
                qb[q] += 1.0
                xt = xin.tile([128, EDGE_TILE], bf16, tag="solo")
                qmap[q].dma_start(
                    out=xt[:],
                    in_=AP(tensor=xs_d, offset=i * EDGE_TILE,
                           ap=[[E_pad, 128], [1, EDGE_TILE]]),
                )
                work.append((xt[:], i * CPT, CPT))
            oq = min(qb, key=qb.get)                # outputs on lighter queue

            # --- compute + overlapped left-half phase B/C ---
            trig = False
            for view, slot0, nsl in work:
                process(view, slot0, nsl)
                if not trig and slot0 + nsl >= LW:
                    trig = True
                    seg_denom(0, LW, 0, SPL, oq)
            seg_denom(RW0, J, SPL, J, oq)

    nc.compile()
    return nc


def _host_prep(x, W, b, index):
    """Sort/pad/bin-pack/shard on host; per-core in_maps + reassembly info."""
    x = np.ascontiguousarray(np.asarray(x, dtype=np.float32))
    W = np.asarray(W, dtype=np.float32).reshape(D)
    b = np.asarray(b, dtype=np.float32).reshape(1)
    idx = np.asarray(index).astype(np.int64).ravel()
    E = idx.shape[0]

    order = np.argsort(idx, kind="stable")
    idx_s = idx[order]
    counts = np.bincount(idx_s, minlength=N_NODES).astype(np.int64)
    seg_starts = np.zeros(N_NODES + 1, dtype=np.int64)
    np.cumsum(counts, out=seg_starts[1:])

    core_e = seg_starts[np.arange(N_CORES + 1) * SEG_PER_CORE]

    # the split-window scans assume every segment spans <= HSL slots
    assert int(counts.max()) <= HSL, f"segment length {counts.max()} > {HSL}"

    # per-core first-fit-decreasing packing of segments (no padding, G=1)
    # into 128 partitions of J slots; J grows in EDGE_TILE/128 steps
    J = 800
    packs = None
    while True:
        packs = []
        ok = True
        for k in range(N_CORES):
            s0 = k * SEG_PER_CORE
            pl = counts[s0:s0 + SEG_PER_CORE]
            sord = np.argsort(pl, kind="stable")[::-1]     # big first
            binid = np.empty(SEG_PER_CORE, dtype=np.int64)
            off = np.empty(SEG_PER_CORE, dtype=np.int64)
            rem = np.full(128, J, dtype=np.int64)
            for s in sord:
                L = int(pl[s])
                bi = int(np.argmax(rem >= L))
                if rem[bi] < L:
                    ok = False
                    break
                binid[s] = bi
                off[s] = J - rem[bi]
                rem[bi] -= L
            if not ok:
                break
            packs.append((binid, off))
        if ok:
            break
        J += EDGE_TILE // 128  # keep E_pad % EDGE_TILE == 0

    E_pad = 128 * J
    x_sorted = x[order]
    wcol = W.reshape(128, 1).astype(BF16)
    bvec = b.reshape(1, 1).astype(np.float32)
    b02 = (NEG_SLOPE * b).reshape(1, 1).astype(np.float32)
    wsq = float(W @ W)
    dummy_col = ((DUMMY_Z / max(wsq, 1e-30)) * W).astype(BF16)  # logit ~ -500

    # padded position P = p*J + t*CPT + c  ->  xsT column t*EDGE_TILE + c*128 + p
    Pv = np.arange(E_pad, dtype=np.int64)
    colmap = ((Pv % J) // CPT) * EDGE_TILE + (Pv % CPT) * 128 + (Pv // J)

    in_maps = []
    reasm = []
    for k in range(N_CORES):
        e0, e1 = int(core_e[k]), int(core_e[k + 1])
        cnt = e1 - e0
        s0 = k * SEG_PER_CORE
        binid, off = packs[k]
        sstart = seg_starts[s0:s0 + SEG_PER_CORE] - e0     # compact local starts

        seg_local = (idx_s[e0:e1] - s0).astype(np.int64)
        pos_in_seg = np.arange(cnt, dtype=np.int64) - sstart[seg_local]
        ppos = binid[seg_local] * J + off[seg_local] + pos_in_seg

        xst = np.empty((128, E_pad), dtype=BF16)
        xst[:] = dummy_col[:, None]
        xst[:, colmap[ppos]] = x_sorted[e0:e1].astype(BF16).T

        # per-slot segment id (unique ids for dummy slots)
        sseg = np.full(128 * J, -1, dtype=np.int64)
        pl = counts[s0:s0 + SEG_PER_CORE]
        slot0 = binid * J + off
        rep_seg = np.repeat(np.arange(SEG_PER_CORE), pl)
        rep_slot = np.repeat(slot0, pl) + (
            np.arange(int(pl.sum()), dtype=np.int64)
            - np.repeat(np.cumsum(pl) - pl, pl))
        sseg[rep_slot] = rep_seg
        dummy_mask = sseg < 0
        sseg[dummy_mask] = SEG_PER_CORE + np.arange(int(dummy_mask.sum()))
        sseg2 = sseg.reshape(128, J)
        notstart = np.ones((128, J), np.float32)
        notstart[:, 1:] = (sseg2[:, 1:] == sseg2[:, :-1])
        notstart[:, 0] = 0.0
        notend = np.ones((128, J), np.float32)
        notend[:, :-1] = (sseg2[:, :-1] == sseg2[:, 1:])
        notend[:, -1] = 0.0

        in_maps.append({
            "xst": xst, "wcol": wcol, "bvec": bvec, "b02": b02,
            "nmask": np.ascontiguousarray(
                np.stack([notstart, notend], axis=1)).astype(BF16),
        })
        reasm.append(ppos)

    return in_maps, reasm, order, core_e, E_pad, E


def _emulate_core(m, E_pad):
    """Numpy emulation of the device graph for one core (host-logic check)."""
    xst, wcol, bvec = m["xst"], m["wcol"], m["bvec"]
    nsm = m["nmask"][:, 0, :].astype(np.float32)
    nem = m["nmask"][:, 1, :].astype(np.float32)
    J = E_pad // 128
    z_cols = (xst.astype(np.float32).T @ wcol.astype(np.float32)).ravel()
    cv = np.arange(E_pad, dtype=np.int64)
    t, rc = cv // EDGE_TILE, cv % EDGE_TILE
    c, p = rc // 128, rc % 128
    P = p * J + t * CPT + c
    z = np.empty(E_pad, dtype=np.float32)
    z[P] = z_cols
    b0 = bvec.ravel()[0]
    zb = z + b0
    l = np.where(zb >= 0, zb, NEG_SLOPE * zb)
    e = np.exp(l).astype(np.float32).reshape(128, J)
    # emulate the split-window scans exactly as the device does
    SPL = ((J // 2) // CPT) * CPT
    LW = SPL + HSL
    RW0 = SPL - HSL
    D4 = np.zeros((128, J), np.float32)
    for (w0, w1, d0, d1) in [(0, LW, 0, SPL), (RW0, J, SPL, J)]:
        fwd = np.zeros((128, w1 - w0), np.float32)
        st = np.zeros(128, np.float32)
        for tt in range(w1 - w0):
            st = nsm[:, w0 + tt] * st + e[:, w0 + tt]
            fwd[:, tt] = st
        d4 = np.zeros((128, w1 - w0), np.float32)
        st = np.zeros(128, np.float32)
        for tt in range(w1 - w0 - 1, -1, -1):
            st = np.maximum(nem[:, w0 + tt] * st, fwd[:, tt])
            d4[:, tt] = st
        D4[:, d0:d1] = d4[:, d0 - w0:d1 - w0]
    R4 = 1.0 / (D4 + 1e-12)
    out = e * R4
    return out.reshape(-1).astype(np.float32)


LAST_RESULTS = None  # BassKernelResults from the most recent run


def kernel(x, W, b, index):
    global LAST_RESULTS
    in_maps, reasm, order, core_e, E_pad, E = _host_prep(x, W, b, index)

    if os.environ.get("KERNEL_EMULATE"):
        outs = [_emulate_core(m, E_pad) for m in in_maps]
    else:
        from concourse.bass_utils import run_bass_kernel_spmd

        if E_pad not in _compiled_cache:
            _compiled_cache[E_pad] = _build_graph(E_pad)
        nc = _compiled_cache[E_pad]
        trace = bool(os.environ.get("BASS_TRACE"))
        LAST_RESULTS = run_bass_kernel_spmd(
            nc, in_maps, list(range(N_CORES)), trace=trace,
        )
        outs = [r["out"] for r in LAST_RESULTS.results]

    out_sorted = np.empty(E, dtype=np.float32)
    for k in range(N_CORES):
        e0, e1 = int(core_e[k]), int(core_e[k + 1])
        out_sorted[e0:e1] = np.asarray(outs[k]).ravel()[reasm[k]]
    out = np.empty(E, dtype=np.float32)
    out[order] = out_sorted
    return out[:, None]
